# revision 6
# baseline (speedup 1.0000x reference)
"""Self-contained Trainium2 kernel for nn_AMDOptimizedAttention.

Reference computes, for B=2, S=2048, H=2048, nh=16, hd=128:
    q/k/v = hs @ w{q,k,v}.T  (torch Linear convention)
    q, k  = rope(q), rope(k)
    out   = causal_softmax(q @ k.T / sqrt(hd)) @ v
    y     = out @ wo.T

Sharding (Megatron-style tensor parallel over heads + data parallel over
batch): core c handles batch c//4, heads 4*(c%4) .. 4*(c%4)+3.  Each core
computes a partial y for its batch (row-sharded wo); host sums the 4
partials per batch (the "all-reduce" is done on host since kernel() must
return the full output anyway).

Device layout notes:
  - all inputs host-pre-transposed so matmul contraction (hidden / head
    dim / key pos) always lands on the SBUF partition axis.
  - scores are computed transposed [k, q]: softmax sum over k is done with
    a ones-vector matmul accumulated in PSUM alongside the PV matmul.
  - matmuls run in float32r (fp22 multiply, fp32 accumulate) which is
    full PE rate for free dim >= 256.
"""

import sys

if "/opt/trn_rl_repo" not in sys.path:
    sys.path.insert(0, "/opt/trn_rl_repo")

import numpy as np

B, S, H = 2, 2048, 2048
NH, HD = 16, 128
P = 128
NCORES = 8
HPC = 4              # heads per core
DSL = HPC * HD       # 512: per-core slice of the hidden dim
KO = H // P          # 16 contraction chunks for projections
TB = 256             # projection token-block (free dim of q/k proj matmuls)
QB = 512             # attention query-block
NTB = S // TB        # 8
NQB = S // QB        # 4
SCALE = 1.0 / np.sqrt(HD)
ROPE_BASE = 10000.0
NEG = -1.0e30

_CACHE = {}


def _build_nc():
    import concourse.mybir as mybir
    from concourse import bacc
    from concourse.tile import TileContext

    f32 = mybir.dt.float32
    f32r = mybir.dt.float32r
    bf16 = mybir.dt.bfloat16
    Alu = mybir.AluOpType
    Act = mybir.ActivationFunctionType

    nc = bacc.Bacc("TRN2", target_bir_lowering=False)

    xT = nc.declare_dram_parameter("xT", [H, S], f32r, isOutput=False)
    wqT = nc.declare_dram_parameter("wqT", [H, DSL], f32r, isOutput=False)
    wkT = nc.declare_dram_parameter("wkT", [H, DSL], f32r, isOutput=False)
    wvT = nc.declare_dram_parameter("wvT", [H, DSL], f32r, isOutput=False)
    woT = nc.declare_dram_parameter("woT", [DSL, H], bf16, isOutput=False)
    cosq = nc.declare_dram_parameter("cosq", [HD // 2, S], f32, isOutput=False)
    sinq = nc.declare_dram_parameter("sinq", [HD // 2, S], f32, isOutput=False)
    cosk = nc.declare_dram_parameter("cosk", [HD // 2, S], f32, isOutput=False)
    sink = nc.declare_dram_parameter("sink", [HD // 2, S], f32, isOutput=False)
    maskp = nc.declare_dram_parameter("mask", [QB // P, P, QB], f32, isOutput=False)
    onesp = nc.declare_dram_parameter("ones", [P, P], f32r, isOutput=False)
    yout = nc.declare_dram_parameter("out", [S, H], f32, isOutput=True)

    xTr = xT.rearrange("(ko p) t -> p ko t", p=P)
    wT = {"q": wqT, "k": wkT, "v": wvT}
    wTr = {k: v.rearrange("(ko p) d -> p ko d", p=P) for k, v in wT.items()}

    def mm(ps, lhsT, rhs, start, stop):
        nc.tensor.matmul(ps, lhsT, rhs, start=start, stop=stop)

    with TileContext(nc) as tc, nc.allow_low_precision(
        reason="f32r/bf16 staging is deliberate; matmuls accumulate in f32 PSUM"
    ):
        with (
            tc.tile_pool(name="resident", bufs=1) as rpool,
            tc.tile_pool(name="spill", bufs=1, space="DRAM") as dpool,
        ):
            # attention output, transposed [d, t], one tile per head
            ao = [rpool.tile([P, S], bf16, tag=f"ao{h}", name=f"ao{h}") for h in range(HPC)]
            # DRAM spill for q/k (transposed, per head) and v (natural)
            qTd = dpool.tile([HPC, P, S], f32r, tag="qTd", name="qTd")
            kTd = dpool.tile([HPC, P, S], f32r, tag="kTd", name="kTd")
            vd = dpool.tile([S, DSL], f32r, tag="vd", name="vd")

            # ---------------- Phase A: projections + rope ----------------
            with (
                tc.tile_pool(name="wres", bufs=1) as wpool,
                tc.tile_pool(name="tabs", bufs=1) as tpool,
                tc.tile_pool(name="xstream", bufs=2) as xpool,
                tc.tile_pool(name="ropetmp", bufs=4) as rtpool,
                tc.tile_pool(name="ropeout", bufs=2) as ropool,
                tc.tile_pool(name="vstage", bufs=2) as vspool,
                tc.tile_pool(name="psqk", bufs=4, space="PSUM") as psqk,
                tc.tile_pool(name="psv", bufs=2, space="PSUM") as psv,
            ):
                ws = {}
                for pj in ("q", "k", "v"):
                    ws[pj] = wpool.tile([P, KO, DSL], f32r, tag=f"w{pj}", name=f"w{pj}")
                    nc.sync.dma_start(ws[pj][:], wTr[pj][:])
                tabs = {}
                for nm, prm in (
                    ("cq", cosq), ("sq", sinq), ("ck", cosk), ("sk", sink)
                ):
                    tabs[nm] = tpool.tile([HD // 2, S], f32, tag=nm, name=nm)
                    nc.sync.dma_start(tabs[nm][:], prm[:])

                for tb in range(NTB):
                    tsl = slice(tb * TB, (tb + 1) * TB)
                    xblk = xpool.tile([P, KO, TB], f32r, tag="xblk", name="xblk")
                    nc.sync.dma_start(xblk[:], xTr[:, :, tsl])

                    for h in range(HPC):
                        dsl = slice(h * P, (h + 1) * P)
                        for pj, dst, ctab, stab in (
                            ("q", qTd, tabs["cq"], tabs["sq"]),
                            ("k", kTd, tabs["ck"], tabs["sk"]),
                        ):
                            ps = psqk.tile([P, TB], f32, tag="psqk", name="psqk")
                            for ko in range(KO):
                                mm(ps, ws[pj][:, ko, dsl], xblk[:, ko, :],
                                   ko == 0, ko == KO - 1)
                            # rope: rows 0:64 = x1*cos - x2*sin,
                            #       rows 64:128 = x1*sin + x2*cos
                            cb = ctab[:, tsl]
                            sb = stab[:, tsl]
                            ro = ropool.tile([P, TB], f32r, tag="ro", name="ro")
                            t1 = rtpool.tile([HD // 2, TB], f32, tag="t1", name="t1")
                            t2 = rtpool.tile([HD // 2, TB], f32, tag="t2", name="t2")
                            nc.vector.tensor_tensor(t1[:], ps[0:64, :], cb, Alu.mult)
                            nc.vector.tensor_tensor(t2[:], ps[64:128, :], sb, Alu.mult)
                            nc.vector.tensor_tensor(ro[0:64, :], t1[:], t2[:], Alu.subtract)
                            nc.vector.tensor_tensor(t1[:], ps[0:64, :], sb, Alu.mult)
                            nc.vector.tensor_tensor(t2[:], ps[64:128, :], cb, Alu.mult)
                            nc.vector.tensor_tensor(ro[64:128, :], t1[:], t2[:], Alu.add)
                            nc.sync.dma_start(dst[h, :, tsl], ro[:])

                    # v in natural layout [t, d]: swap matmul operands
                    for mi in range(TB // P):
                        t0 = tb * TB + mi * P
                        pv = psv.tile([P, DSL], f32, tag="psv", name="psv")
                        for ko in range(KO):
                            mm(pv, xblk[:, ko, mi * P:(mi + 1) * P], ws["v"][:, ko, :],
                               ko == 0, ko == KO - 1)
                        vst = vspool.tile([P, DSL], f32r, tag="vst", name="vst")
                        nc.vector.tensor_copy(vst[:], pv[:])
                        nc.sync.dma_start(vd[t0:t0 + P, :], vst[:])

            # ---------------- Phase B: attention ----------------
            with (
                tc.tile_pool(name="battn", bufs=1) as bpool,
                tc.tile_pool(name="kvres", bufs=2) as kvpool,
                tc.tile_pool(name="qblk", bufs=3) as qpool,
                tc.tile_pool(name="et", bufs=4) as epool,
                tc.tile_pool(name="rec", bufs=2) as recpool,
                tc.tile_pool(name="pss", bufs=2, space="PSUM") as pss,
                tc.tile_pool(name="pso", bufs=2, space="PSUM") as pso,
                tc.tile_pool(name="psd", bufs=2, space="PSUM") as psd,
                tc.tile_pool(name="psb", bufs=2, space="PSUM") as psbp,
            ):
                masks = bpool.tile([P, QB // P, QB], f32, tag="masks", name="masks")
                nc.sync.dma_start(masks[:], maskp.rearrange("j p f -> p j f"))
                ones = bpool.tile([P, P], f32r, tag="ones", name="ones")
                nc.sync.dma_start(ones[:], onesp[:])
                vdr = vd[:].rearrange("(ko p) d -> p ko d", p=P)

                for h in range(HPC):
                    kh = kvpool.tile([P, S], f32r, tag="kh", name="kh")
                    nc.sync.dma_start(kh[:], kTd[h])
                    vh = kvpool.tile([P, S // P, P], f32r, tag="vh", name="vh")
                    nc.sync.dma_start(vh[:], vdr[:, :, h * P:(h + 1) * P])

                    for qb in range(NQB):
                        qsl = slice(qb * QB, (qb + 1) * QB)
                        qblk = qpool.tile([P, QB], f32r, tag="qblk", name="qblk")
                        nc.sync.dma_start(qblk[:], qTd[h][:, qsl])
                        nkt = (qb + 1) * (QB // P)
                        po = pso.tile([P, QB], f32, tag="po", name="po")
                        pd = psd.tile([1, QB], f32, tag="pd", name="pd")
                        for kt in range(nkt):
                            pscr = pss.tile([P, QB], f32, tag="pscr", name="pscr")
                            mm(pscr, kh[:, kt * P:(kt + 1) * P], qblk[:],
                               True, True)
                            j = kt - qb * (QB // P)
                            if j >= 0:
                                nc.vector.tensor_tensor(
                                    pscr[:], pscr[:], masks[:, j, :], Alu.add
                                )
                            et = epool.tile([P, QB], f32r, tag="et", name="et")
                            nc.scalar.activation(et[:], pscr[:], Act.Exp)
                            mm(po, vh[:, kt, :], et[:], kt == 0, kt == nkt - 1)
                            mm(pd, ones[:, 0:1], et[:], kt == 0, kt == nkt - 1)
                        rec = recpool.tile([1, QB], f32r, tag="rec", name="rec")
                        nc.vector.reciprocal(rec[:], pd[:])
                        pb = psbp.tile([P, QB], f32, tag="pb", name="pb")
                        mm(pb, ones[0:1, :], rec[:], True, True)
                        pbs = recpool.tile([P, QB], f32, tag="pbs", name="pbs")
                        nc.vector.tensor_copy(pbs[:], pb[:])
                        nc.vector.tensor_tensor(
                            ao[h][:, qsl], po[:], pbs[:], Alu.mult
                        )

            # ---------------- Phase C: output projection ----------------
            with (
                tc.tile_pool(name="wo", bufs=1) as wopool,
                tc.tile_pool(name="ystage", bufs=2) as ypool,
                tc.tile_pool(name="psy", bufs=4, space="PSUM") as psy,
            ):
                wos = wopool.tile([P, DSL // P, H], bf16, tag="wos", name="wos")
                nc.sync.dma_start(wos[:], woT.rearrange("(ko p) e -> p ko e", p=P))
                for tt in range(S // P):
                    tsl = slice(tt * P, (tt + 1) * P)
                    yst = ypool.tile([P, H], f32, tag="yst", name="yst")
                    for ec in range(H // QB):
                        py = psy.tile([P, QB], f32, tag="py", name="py")
                        for dc in range(DSL // P):
                            mm(py, ao[dc][:, tsl], wos[:, dc, ec * QB:(ec + 1) * QB],
                               dc == 0, dc == DSL // P - 1)
                        nc.vector.tensor_copy(yst[:, ec * QB:(ec + 1) * QB], py[:])
                    nc.sync.dma_start(yout[tsl, :], yst[:])

    nc.finalize()
    return nc


def _bf16np():
    import ml_dtypes
    return ml_dtypes.bfloat16


def _host_inputs(hidden_states, wq, wk, wv, wo):
    f32 = np.float32
    ca = np.ascontiguousarray

    inv = 1.0 / (ROPE_BASE ** (np.arange(0, HD, 2, dtype=f32) / HD))
    t = np.arange(S, dtype=f32)
    fr = np.outer(t, inv)                      # [S, 64]
    cosT = ca(np.cos(fr).T.astype(f32))        # [64, S]
    sinT = ca(np.sin(fr).T.astype(f32))
    cosq = ca(cosT * f32(SCALE))
    sinq = ca(sinT * f32(SCALE))

    jj, pp, ff = np.meshgrid(
        np.arange(QB // P), np.arange(P), np.arange(QB), indexing="ij"
    )
    mask = np.where(jj * P + pp > ff, f32(NEG), f32(0.0)).astype(f32)
    ones = np.ones((P, P), f32)

    xTb = [ca(hidden_states[b].T.astype(f32)) for b in range(B)]

    in_maps = []
    for c in range(NCORES):
        b, hg = divmod(c, NCORES // B)
        dsl = slice(hg * DSL, (hg + 1) * DSL)
        in_maps.append({
            "xT": xTb[b],
            "wqT": ca(wq[dsl, :].T.astype(f32)),
            "wkT": ca(wk[dsl, :].T.astype(f32)),
            "wvT": ca(wv[dsl, :].T.astype(f32)),
            "woT": ca(wo[:, dsl].T.astype(_bf16np())),
            "cosq": cosq, "sinq": sinq, "cosk": cosT, "sink": sinT,
            "mask": mask, "ones": ones,
        })
    return in_maps


def kernel(hidden_states, wq, wk, wv, wo, trace=False):
    from concourse.bass_utils import run_bass_kernel_spmd

    if "nc" not in _CACHE:
        _CACHE["nc"] = _build_nc()
    nc = _CACHE["nc"]

    in_maps = _host_inputs(
        np.asarray(hidden_states), np.asarray(wq), np.asarray(wk),
        np.asarray(wv), np.asarray(wo),
    )
    res = run_bass_kernel_spmd(nc, in_maps, core_ids=list(range(NCORES)),
                               trace=trace)
    y = np.zeros((B, S, H), np.float32)
    for c in range(NCORES):
        y[c // (NCORES // B)] += res.results[c]["out"]
    if trace:
        return y, res
    return y
